# revision 6
# baseline (speedup 1.0000x reference)
"""BitLinear158 Trainium2 kernel — fp8 DoubleRow, partial hi/lo correction,
sharded-gamma AllReduce prologue.

Reference computation:
    gamma = mean(|W|)
    Wq    = clip(round(W / (gamma + 1e-5)), -1, 1)      # ternary {-1, 0, +1}
    out   = x @ Wq.T + b                                # x: [8, 4096, 2048]

Sharding: data-parallel over the batch dim (8 batches -> 8 cores) for the
GEMM; the gamma reduction is sharded over cores (each core reduces a
host-sliced 2 MiB slice of W, partial column-sums are AllReduced as a
[128,1] f32 vector through a DRAM bounce), so thresholds are known ~15us
in instead of after a full serial 16 MiB W read.

Math: Wq is ternary so it is EXACT in fp8e4 (e4m3). The fp8 DoubleRow
matmul contracts K=256 per instruction at the same per-instruction cost as
a bf16 K=128 matmul (measured 216ns at 512 free rows) -> 2x FLOP rate.
Activations split x = hi + lo, hi = fp8(x), lo = fp8(x - hi); hi covers all
16 k-tiles, lo corrects the last 2L (L of 8 k-pairs). Output L2 rel error
= 2.35e-2 * sqrt(1 - L/8); L=4 -> 1.66e-2 measured (gate 2e-2).

Device pipeline per core:
  gamma:   2 MiB shard -> |.| partial sums (DVE+ACT split) -> [128,1]
           AllReduce -> ones-matmul partition reduce -> thresholds
           +-0.5*(gamma+eps).
  quant:   single 16 MiB W stream; ternarize via (W > thr) + (W >= -thr) - 1
           (2 ops/tile, even tiles on DVE, odd on GpSimd) into a resident
           fp8e4 WqT tensor [128, 16, 2048].
  main:    epochs of 2 token-tiles x 4 output chunks = 8 concurrent
           [128,512] PSUM groups; per token tile: fp32 x DMA (prefetched
           during the prologue), ACT casts hi, DVE computes lo for the
           corrected k-range; 8 hi + L lo DoubleRow matmuls per group;
           bias-add fused into PSUM->SBUF eviction on DVE; fp32 out.
"""

from contextlib import ExitStack

import numpy as np

import concourse.bacc as bacc
import concourse.bass as bass
import concourse.mybir as mybir
import concourse.tile as tile
from concourse.bass_utils import run_bass_kernel_spmd

P = 128
B, S, D_IN, D_OUT = 8, 4096, 2048, 2048
N_CORES = 8
TOK = (B * S) // N_CORES          # 4096 tokens per core
KT = D_IN // P                    # 16 k-tiles
KK = KT // 2                      # 8 k-pairs (DoubleRow contracts 2 tiles)
L = 4                             # k-pairs receiving the lo correction
TT = TOK // P                     # 32 token tiles
NC_CHUNK = 512                    # matmul moving free dim (1 PSUM bank fp32)
OC = D_OUT // NC_CHUNK            # 4 output chunks
W_ELEMS = D_OUT * D_IN            # 2**22 (power of 2: S/N == S*(1/N) exactly)
EPS = 1e-5
CKP0 = 0                          # first corrected k-pair
SH_ROWS = D_IN // N_CORES         # 256 k-rows of W per core's gamma shard
PRE_EP = 2                        # epochs of x prefetched during prologue

F32 = mybir.dt.float32
BF16 = mybir.dt.bfloat16
FP8 = mybir.dt.float8e4
DR = mybir.MatmulPerfMode.DoubleRow
MULT = mybir.AluOpType.mult
ADD = mybir.AluOpType.add
IS_GT = mybir.AluOpType.is_gt
IS_GE = mybir.AluOpType.is_ge
AX_X = mybir.AxisListType.X


def build_nc() -> bass.Bass:
    nc = bacc.Bacc(None, target_bir_lowering=False)
    xT = nc.dram_tensor("xT", [D_IN, TOK], F32, kind="ExternalInput")
    WT = nc.dram_tensor("WT", [D_IN, D_OUT], F32, kind="ExternalInput")
    Wsh = nc.dram_tensor("Wsh", [SH_ROWS, D_OUT], F32, kind="ExternalInput")
    b = nc.dram_tensor("b", [D_OUT], F32, kind="ExternalInput")
    out = nc.dram_tensor("out", [TOK, D_OUT], F32, kind="ExternalOutput")

    with tile.TileContext(nc) as tc, ExitStack() as ctx:
        wpool = ctx.enter_context(tc.tile_pool(name="wpass", bufs=4))
        spool = ctx.enter_context(tc.tile_pool(name="scalars", bufs=1))
        qpool_v = ctx.enter_context(tc.tile_pool(name="qtmpv", bufs=2))
        wqpool = ctx.enter_context(tc.tile_pool(name="wq", bufs=1))
        xfpool = ctx.enter_context(tc.tile_pool(name="xf", bufs=4))
        xhpool = ctx.enter_context(tc.tile_pool(name="xh", bufs=4))
        xlpool = ctx.enter_context(tc.tile_pool(name="xl", bufs=4))
        opool = ctx.enter_context(tc.tile_pool(name="osb", bufs=3))
        dpool = ctx.enter_context(tc.tile_pool(name="dram", bufs=2, space="DRAM"))
        pspool = ctx.enter_context(
            tc.tile_pool(name="psum", bufs=8, space="PSUM")
        )

        xT_v = xT.rearrange("(a p) t -> p a t", p=P)  # [128, KT, TOK]

        # ---- x prefetch for the first PRE_EP epochs: issued first on the
        # gpsimd queue so the DMAs flow while the gamma shard reduces.
        xfs = {}
        for tt in range(2 * PRE_EP):
            xf = xfpool.tile([P, KT, P], F32, tag="xf", name=f"xf{tt}")
            nc.gpsimd.dma_start(xf[:], xT_v[:, :, tt * P : (tt + 1) * P])
            xfs[tt] = xf

        # ---- sharded gamma: this core reduces its 2 MiB W slice ----
        wsh_v = Wsh.rearrange("(a p) o -> p a o", p=P)  # [128, 2, D_OUT]
        wsh = spool.tile([P, 2, D_OUT], F32)
        nc.sync.dma_start(wsh[:], wsh_v[:])
        pd = spool.tile([P, 1], F32)
        pa = spool.tile([P, 1], F32)
        actdump = spool.tile([P, D_OUT], BF16)
        nc.vector.reduce_sum(
            pd[:], wsh[:, 0, :], axis=AX_X, apply_absolute_value=True
        )
        nc.scalar.activation(
            actdump[:],
            wsh[:, 1, :],
            mybir.ActivationFunctionType.Abs,
            accum_out=pa[:],
        )
        colsum_loc = spool.tile([P, 1], F32)
        nc.vector.tensor_add(colsum_loc[:], pd[:], pa[:])

        # AllReduce the [128,1] partial across the 8 cores (DRAM bounce).
        cc_in = dpool.tile([P, 1], F32)
        cc_out = dpool.tile([P, 1], F32)
        nc.gpsimd.dma_start(cc_in[:], colsum_loc[:])
        nc.gpsimd.collective_compute(
            "AllReduce",
            ADD,
            replica_groups=[list(range(N_CORES))],
            ins=[cc_in[:].opt()],
            outs=[cc_out[:].opt()],
        )
        colsum_g = spool.tile([P, 1], F32)
        nc.gpsimd.dma_start(colsum_g[:], cc_out[:])

        # Bias replicated to all partitions; on the gpsimd queue so it does
        # not delay the W stream on sync.
        bias_sb = spool.tile([P, D_OUT], F32)
        b_row = b[:].rearrange("(o d) -> o d", o=1)
        nc.gpsimd.dma_start(bias_sb[:], b_row.to_broadcast((P, D_OUT)))

        # Partition reduce + broadcast in one PE op: ones.T @ colsum_g puts
        # sum over partitions on every partition.
        ones_sq = spool.tile([P, P], F32)
        nc.vector.memset(ones_sq[:], 1.0)
        total_ps = pspool.tile([P, NC_CHUNK], F32, tag="ps")
        nc.tensor.matmul(
            total_ps[:, 0:1], ones_sq[:], colsum_g[:], start=True, stop=True
        )

        # Quantization thresholds: W > thr  <=>  W/(gamma+eps) > 0.5.
        geps = spool.tile([P, 1], F32)
        nc.vector.tensor_scalar(
            geps[:], total_ps[:, 0:1], 1.0 / W_ELEMS, EPS, MULT, ADD
        )
        thr = spool.tile([P, 1], F32)
        nc.vector.tensor_scalar_mul(thr[:], geps[:], 0.5)
        negthr = spool.tile([P, 1], F32)
        nc.vector.tensor_scalar_mul(negthr[:], geps[:], -0.5)

        # ---- hi/lo split for the prefetched token tiles (ACT + DVE) ----
        xhs, xls = {}, {}
        for tt in range(2 * PRE_EP):
            xh = xhpool.tile([P, KT, P], FP8, tag="xh", name=f"xh{tt}")
            nc.scalar.activation(
                xh[:], xfs[tt][:], mybir.ActivationFunctionType.Copy
            )
            xhs[tt] = xh
        for tt in range(2 * PRE_EP):
            xl = xlpool.tile([P, 2 * L, P], FP8, tag="xl", name=f"xl{tt}")
            nc.vector.tensor_sub(
                xl[:],
                xfs[tt][:, 2 * CKP0 : 2 * (CKP0 + L), :],
                xhs[tt][:, 2 * CKP0 : 2 * (CKP0 + L), :],
            )
            xls[tt] = xl

        # ---- single W stream + quantize (even k-tiles on DVE, odd on
        # GpSimd): WqT = (W > thr) + (W >= -thr) - 1 in {-1, 0, +1},
        # exact in fp8e4.
        wq8 = wqpool.tile([P, KT, D_OUT], FP8)
        for kt in range(KT):
            wt = wpool.tile([P, D_OUT], F32, tag="wt", name=f"w_{kt}")
            nc.sync.dma_start(wt[:], WT[kt * P : (kt + 1) * P, :])
            ga = qpool_v.tile([P, D_OUT], FP8, tag="q")
            nc.vector.tensor_scalar(ga[:], wt[:], thr[:], -1.0, IS_GT, ADD)
            nc.vector.scalar_tensor_tensor(
                wq8[:, kt, :], wt[:], negthr[:], ga[:], IS_GE, ADD
            )

        LO_SET = list(range(CKP0, CKP0 + L))  # corrected k-pairs

        # ---- main: out[t, :] = x[t, :] @ WqT + b ----
        TPE = 2  # token tiles per epoch
        for ep in range(TT // TPE):
            for i in range(TPE):
                tt = ep * TPE + i
                if tt in xfs:
                    continue
                xf = xfpool.tile([P, KT, P], F32, tag="xf", name=f"xf{tt}")
                nc.gpsimd.dma_start(xf[:], xT_v[:, :, tt * P : (tt + 1) * P])
                xh = xhpool.tile([P, KT, P], FP8, tag="xh", name=f"xh{tt}")
                nc.scalar.activation(
                    xh[:], xf[:], mybir.ActivationFunctionType.Copy
                )
                xl = xlpool.tile([P, 2 * L, P], FP8, tag="xl", name=f"xl{tt}")
                nc.vector.tensor_sub(
                    xl[:],
                    xf[:, 2 * CKP0 : 2 * (CKP0 + L), :],
                    xh[:, 2 * CKP0 : 2 * (CKP0 + L), :],
                )
                xfs[tt], xhs[tt], xls[tt] = xf, xh, xl

            groups = [(i, oc) for i in range(TPE) for oc in range(OC)]
            pss = [
                pspool.tile([P, NC_CHUNK], F32, tag="ps", name=f"ps{g}")
                for g in range(len(groups))
            ]
            for ki in range(KK):
                for g, (i, oc) in enumerate(groups):
                    tt = ep * TPE + i
                    nc.tensor.matmul(
                        pss[g][:],
                        xhs[tt][:, 2 * ki : 2 * ki + 2, :],
                        wq8[:, 2 * ki : 2 * ki + 2,
                            oc * NC_CHUNK : (oc + 1) * NC_CHUNK],
                        start=(ki == 0),
                        stop=False,
                        perf_mode=DR,
                    )
            for li, kkp in enumerate(LO_SET):
                for g, (i, oc) in enumerate(groups):
                    tt = ep * TPE + i
                    nc.tensor.matmul(
                        pss[g][:],
                        xls[tt][:, 2 * (kkp - CKP0) : 2 * (kkp - CKP0) + 2, :],
                        wq8[:, 2 * kkp : 2 * kkp + 2,
                            oc * NC_CHUNK : (oc + 1) * NC_CHUNK],
                        start=False,
                        stop=(li == L - 1),
                        perf_mode=DR,
                    )

            for i in range(TPE):
                tt = ep * TPE + i
                osb = opool.tile([P, D_OUT], F32, tag="osb")
                for oc in range(OC):
                    nc.vector.tensor_add(
                        osb[:, oc * NC_CHUNK : (oc + 1) * NC_CHUNK],
                        pss[i * OC + oc][:],
                        bias_sb[:, oc * NC_CHUNK : (oc + 1) * NC_CHUNK],
                    )
                nc.sync.dma_start(out[tt * P : (tt + 1) * P, :], osb[:])
                del xfs[tt], xhs[tt], xls[tt]

    nc.finalize()
    return nc


_NC_CACHE: list = []


def _get_nc() -> bass.Bass:
    if not _NC_CACHE:
        _NC_CACHE.append(build_nc())
    return _NC_CACHE[0]


def make_in_maps(x: np.ndarray, W: np.ndarray, b: np.ndarray):
    x = np.asarray(x, dtype=np.float32).reshape(N_CORES, TOK, D_IN)
    W = np.asarray(W, dtype=np.float32)
    b = np.asarray(b, dtype=np.float32)
    WT = np.ascontiguousarray(W.T)
    return [
        {
            "xT": np.ascontiguousarray(x[c].T),
            "WT": WT,
            "Wsh": np.ascontiguousarray(
                WT[c * SH_ROWS : (c + 1) * SH_ROWS]
            ),
            "b": b,
        }
        for c in range(N_CORES)
    ]


def run(x, W, b, **spmd_kwargs):
    """Run the SPMD kernel; returns (full_output, BassKernelResults)."""
    nc = _get_nc()
    in_maps = make_in_maps(x, W, b)
    res = run_bass_kernel_spmd(nc, in_maps, list(range(N_CORES)), **spmd_kwargs)
    out = np.stack([res.results[c]["out"] for c in range(N_CORES)], axis=0)
    return out.reshape(B, S, D_OUT), res


def kernel(x, W, b):
    out, _ = run(x, W, b)
    return out


# revision 7
# speedup vs baseline: 1.2687x; 1.2687x over previous
"""BitLinear158 Trainium2 kernel — fp8 DoubleRow with partial hi/lo correction.

Reference computation:
    gamma = mean(|W|)
    Wq    = clip(round(W / (gamma + 1e-5)), -1, 1)      # ternary {-1, 0, +1}
    out   = x @ Wq.T + b                                # x: [8, 4096, 2048]

Sharding: data-parallel over the batch dim (8 batches -> 8 cores). Each core
gets x[i] (host-transposed to k-major), the full W (host-transposed) and b.
gamma is computed redundantly per-core -- measured cross-core collective
latency/skew (~80us) far exceeds the 45us it would save.

Math: Wq is ternary so it is EXACT in fp8e4 (e4m3). The fp8 DoubleRow matmul
contracts K=256 per instruction at the same per-instruction cost as a bf16
K=128 matmul (measured 216ns at 512 free rows) -> 2x FLOP rate. Activations
split x = hi + lo with hi = fp8(x), lo = fp8(x - hi); hi covers all 16
k-tiles, lo corrects k-tiles 8..15 (L=4 of 8 k-pairs). Output L2 rel error
= 2.35e-2 * sqrt(1 - L/8) -> 1.66e-2 measured on HW (gate 2e-2).

Device pipeline per core:
  pass 1: stream WT (16 MiB), |.|+row-sum split across DVE and ACT so the
          pass is DMA-bound; ones-matmul partition reduce; thresholds
          +-0.5*(gamma+eps). The last NRET W tiles stay resident.
  pass 2: ternarize via (W > thr) + (W >= -thr) - 1 (two DVE ops/tile) into
          a resident fp8e4 WqT tensor [128,16,2048]; resident k-PAIRS first
          (highest pair down), then re-streamed tiles in descending order so
          k-pairs complete earliest-first.
  main:   epochs of 2 token-tiles x 4 output chunks = 8 concurrent
          [128,512] PSUM groups; per token tile: fp32 x DMA (deferred
          behind pass 1), ACT casts hi, GPSIMD computes lo = fp8(x - hi);
          12 DoubleRow matmuls per group emitted in quantize-completion
          order [hi 7..4, lo 7..4, hi 3..0]; bias-add fused into PSUM
          eviction on DVE; fp32 out.
"""

from contextlib import ExitStack

import numpy as np

import concourse.bacc as bacc
import concourse.bass as bass
import concourse.mybir as mybir
import concourse.tile as tile
from concourse.bass_utils import run_bass_kernel_spmd

P = 128
B, S, D_IN, D_OUT = 8, 4096, 2048, 2048
N_CORES = 8
TOK = (B * S) // N_CORES          # 4096 tokens per core
KT = D_IN // P                    # 16 k-tiles
KK = KT // 2                      # 8 k-pairs (DoubleRow contracts 2 tiles)
L = 4                             # k-pairs receiving the lo correction
TT = TOK // P                    # 32 token tiles
NC_CHUNK = 512                    # matmul moving free dim (1 PSUM bank fp32)
OC = D_OUT // NC_CHUNK            # 4 output chunks
W_ELEMS = D_OUT * D_IN            # 2**22 (power of 2: S/N == S*(1/N) exactly)
EPS = 1e-5
CKP0 = KK - L                     # first corrected k-pair (tiles 8..15)
NRET = 9                          # W tiles retained between pass 1 and quant

F32 = mybir.dt.float32
BF16 = mybir.dt.bfloat16
FP8 = mybir.dt.float8e4
DR = mybir.MatmulPerfMode.DoubleRow
MULT = mybir.AluOpType.mult
ADD = mybir.AluOpType.add
IS_GT = mybir.AluOpType.is_gt
IS_GE = mybir.AluOpType.is_ge
AX_X = mybir.AxisListType.X


def build_nc() -> bass.Bass:
    nc = bacc.Bacc(None, target_bir_lowering=False)
    xT = nc.dram_tensor("xT", [D_IN, TOK], F32, kind="ExternalInput")
    WT = nc.dram_tensor("WT", [D_IN, D_OUT], F32, kind="ExternalInput")
    b = nc.dram_tensor("b", [D_OUT], F32, kind="ExternalInput")
    out = nc.dram_tensor("out", [TOK, D_OUT], F32, kind="ExternalOutput")

    with tile.TileContext(nc) as tc, ExitStack() as ctx:
        wpool = ctx.enter_context(tc.tile_pool(name="wpass", bufs=NRET + 1))
        spool = ctx.enter_context(tc.tile_pool(name="scalars", bufs=1))
        qpool = ctx.enter_context(tc.tile_pool(name="qtmp", bufs=2))
        wqpool = ctx.enter_context(tc.tile_pool(name="wq", bufs=1))
        xfpool = ctx.enter_context(tc.tile_pool(name="xf", bufs=4))
        xhpool = ctx.enter_context(tc.tile_pool(name="xh", bufs=4))
        xlpool = ctx.enter_context(tc.tile_pool(name="xl", bufs=4))
        opool = ctx.enter_context(tc.tile_pool(name="osb", bufs=3))
        pspool = ctx.enter_context(
            tc.tile_pool(name="psum", bufs=8, space="PSUM")
        )

        # ---- pass 1: gamma = mean |W|, |.|+row-sum split DVE/ACT so the
        # pass is DMA-bound. The last NRET tiles stay resident.
        partials_dve = spool.tile([P, KT // 2], F32)
        partials_act = spool.tile([P, KT // 2], F32)
        actdump = qpool.tile([P, D_OUT], BF16, tag="q")
        w_resident = {}
        last_w1_dma = None
        for kt in range(KT):
            wt = wpool.tile([P, D_OUT], F32, tag="wt", name=f"w1_{kt}")
            last_w1_dma = nc.sync.dma_start(wt[:], WT[kt * P : (kt + 1) * P, :])
            if kt % 2 == 0:
                nc.vector.reduce_sum(
                    partials_dve[:, kt // 2 : kt // 2 + 1],
                    wt[:],
                    axis=AX_X,
                    apply_absolute_value=True,
                )
            else:
                nc.scalar.activation(
                    actdump[:],
                    wt[:],
                    mybir.ActivationFunctionType.Abs,
                    accum_out=partials_act[:, kt // 2 : kt // 2 + 1],
                )
            if kt >= KT - NRET:
                w_resident[kt] = wt
        # Bias replicated to all partitions, deferred behind pass 1.
        bias_sb = spool.tile([P, D_OUT], F32)
        b_row = b[:].rearrange("(o d) -> o d", o=1)
        bias_dma = nc.sync.dma_start(bias_sb[:], b_row.to_broadcast((P, D_OUT)))
        tile.add_dep_helper(
            bias_dma.ins, last_w1_dma.ins, reason="defer bias behind pass1"
        )

        c1 = spool.tile([P, 1], F32)
        nc.vector.reduce_sum(c1[:], partials_dve[:], axis=AX_X)
        c2 = spool.tile([P, 1], F32)
        nc.vector.reduce_sum(c2[:], partials_act[:], axis=AX_X)
        colsum = spool.tile([P, 1], F32)
        nc.vector.tensor_add(colsum[:], c1[:], c2[:])

        # Partition reduce + broadcast in one PE op.
        ones_sq = spool.tile([P, P], F32)
        nc.vector.memset(ones_sq[:], 1.0)
        total_ps = pspool.tile([P, NC_CHUNK], F32, tag="ps")
        nc.tensor.matmul(
            total_ps[:, 0:1], ones_sq[:], colsum[:], start=True, stop=True
        )

        # Quantization thresholds: W > thr  <=>  W/(gamma+eps) > 0.5.
        geps = spool.tile([P, 1], F32)
        nc.vector.tensor_scalar(
            geps[:], total_ps[:, 0:1], 1.0 / W_ELEMS, EPS, MULT, ADD
        )
        thr = spool.tile([P, 1], F32)
        nc.vector.tensor_scalar_mul(thr[:], geps[:], 0.5)
        negthr = spool.tile([P, 1], F32)
        nc.vector.tensor_scalar_mul(negthr[:], geps[:], -0.5)

        # ---- pass 2: WqT = (W > thr) + (W >= -thr) - 1 in {-1, 0, +1} ----
        # Resident tiles first in descending pair order, then re-streamed
        # tiles descending so k-pairs complete earliest-first for the ramp.
        K_ORDER = list(range(KT - 1, KT - NRET - 1, -1)) + list(
            range(KT - NRET - 1, -1, -1)
        )
        wq8 = wqpool.tile([P, KT, D_OUT], FP8)
        for kt in K_ORDER:
            if kt in w_resident:
                wt = w_resident[kt]
            else:
                wt = wpool.tile([P, D_OUT], F32, tag="wt", name=f"w2_{kt}")
                nc.sync.dma_start(wt[:], WT[kt * P : (kt + 1) * P, :])
            ga = qpool.tile([P, D_OUT], FP8, tag="q")
            nc.vector.tensor_scalar(ga[:], wt[:], thr[:], -1.0, IS_GT, ADD)
            nc.vector.scalar_tensor_tensor(
                wq8[:, kt, :], wt[:], negthr[:], ga[:], IS_GE, ADD
            )

        # Per-group matmul emission order (matches quantize completion):
        # hi on pairs 7..4 (resident), lo on 7..4, then hi on 3..0.
        MM_ORDER = (
            [("h", kkp) for kkp in range(KK - 1, CKP0 - 1, -1)]
            + [("l", kkp) for kkp in range(KK - 1, CKP0 - 1, -1)]
            + [("h", kkp) for kkp in range(CKP0 - 1, -1, -1)]
        )

        # ---- main: out[t, :] = x[t, :] @ WqT + b ----
        xT_v = xT.rearrange("(a p) t -> p a t", p=P)  # [128, KT, TOK]
        TPE = 2  # token tiles per epoch
        first_xf_dma = True
        for ep in range(TT // TPE):
            xhs, xls = [], []
            for i in range(TPE):
                tt = ep * TPE + i
                xf = xfpool.tile([P, KT, P], F32, tag="xf")
                xf_dma = nc.gpsimd.dma_start(
                    xf[:], xT_v[:, :, tt * P : (tt + 1) * P]
                )
                if first_xf_dma:
                    # x competes with the gamma-critical W stream for HBM;
                    # hold it back until pass 1 is issued.
                    first_xf_dma = False
                    tile.add_dep_helper(
                        xf_dma.ins,
                        last_w1_dma.ins,
                        reason="defer x behind pass1",
                    )
                xh = xhpool.tile([P, KT, P], FP8, tag="xh")
                nc.scalar.activation(
                    xh[:], xf[:], mybir.ActivationFunctionType.Copy
                )
                xl = xlpool.tile([P, 2 * L, P], FP8, tag="xl")
                nc.gpsimd.tensor_sub(
                    xl[:], xf[:, 2 * CKP0 :, :], xh[:, 2 * CKP0 :, :]
                )
                xhs.append(xh)
                xls.append(xl)

            groups = [(i, oc) for i in range(TPE) for oc in range(OC)]
            pss = [
                pspool.tile([P, NC_CHUNK], F32, tag="ps", name=f"ps{g}")
                for g in range(len(groups))
            ]
            for mi, (kind, kkp) in enumerate(MM_ORDER):
                for g, (i, oc) in enumerate(groups):
                    if kind == "h":
                        stat = xhs[i][:, 2 * kkp : 2 * kkp + 2, :]
                    else:
                        stat = xls[i][
                            :, 2 * (kkp - CKP0) : 2 * (kkp - CKP0) + 2, :
                        ]
                    nc.tensor.matmul(
                        pss[g][:],
                        stat,
                        wq8[:, 2 * kkp : 2 * kkp + 2,
                            oc * NC_CHUNK : (oc + 1) * NC_CHUNK],
                        start=(mi == 0),
                        stop=(mi == len(MM_ORDER) - 1),
                        perf_mode=DR,
                    )

            for i in range(TPE):
                tt = ep * TPE + i
                osb = opool.tile([P, D_OUT], F32, tag="osb")
                for oc in range(OC):
                    nc.vector.tensor_add(
                        osb[:, oc * NC_CHUNK : (oc + 1) * NC_CHUNK],
                        pss[i * OC + oc][:],
                        bias_sb[:, oc * NC_CHUNK : (oc + 1) * NC_CHUNK],
                    )
                nc.sync.dma_start(out[tt * P : (tt + 1) * P, :], osb[:])

    nc.finalize()
    return nc


_NC_CACHE: list = []


def _get_nc() -> bass.Bass:
    if not _NC_CACHE:
        _NC_CACHE.append(build_nc())
    return _NC_CACHE[0]


def make_in_maps(x: np.ndarray, W: np.ndarray, b: np.ndarray):
    x = np.asarray(x, dtype=np.float32).reshape(N_CORES, TOK, D_IN)
    W = np.asarray(W, dtype=np.float32)
    b = np.asarray(b, dtype=np.float32)
    WT = np.ascontiguousarray(W.T)
    return [
        {"xT": np.ascontiguousarray(x[c].T), "WT": WT, "b": b}
        for c in range(N_CORES)
    ]


def run(x, W, b, **spmd_kwargs):
    """Run the SPMD kernel; returns (full_output, BassKernelResults)."""
    nc = _get_nc()
    in_maps = make_in_maps(x, W, b)
    res = run_bass_kernel_spmd(nc, in_maps, list(range(N_CORES)), **spmd_kwargs)
    out = np.stack([res.results[c]["out"] for c in range(N_CORES)], axis=0)
    return out.reshape(B, S, D_OUT), res


def kernel(x, W, b):
    out, _ = run(x, W, b)
    return out
